# revision 1
# baseline (speedup 1.0000x reference)
"""AttentionLSTMDecoder Trainium2 kernel (8-core SPMD), v2.

Sharding: data-parallel over batch B=64 -> 8 graphs/core for the
recurrent part (attention over that core's node segment + 2-layer LSTM),
AllGather of h1 trajectories in 4-step chunks (bf16), vocab-sharded fc
(each core computes a 4096-wide padded vocab slice for all positions).

v2 changes vs baseline:
- all matmul operands bf16 (weights, activations, stationaries).
- LSTM cell uses only tanh (sigmoid(x) = (1+tanh(x/2))/2, with the 0.5
  gate scales and a doubled-h state folded into the weights host-side)
  -> sigmoid/exp ACT-table thrash eliminated (exp+tanh share one table).
- gate PSUM as [8,512] quarters, double-buffered -> no WAR stalls, the
  W_hh1 part of layer-1 gates runs during the layer-0 cell.
- b_a folded into the mask row host-side (scores += np@b_a).
- fc restructured: hA chunks as stationary, W_fc as 512-wide moving,
  interleaved into recurrence idle slots, bf16 output, bias on host.
"""

import math

import numpy as np

B, T, H, E, V, NTOT = 64, 20, 512, 512, 32000, 8192
NCORES = 8
BL = B // NCORES          # 8 graphs per core
POS = T * BL              # 160 positions per core
VSH = V // NCORES         # 4000 vocab rows per core
VPAD = 4096               # padded vocab shard
G4 = 4 * H                # 2048 gate width
NEG = -40.0               # mask bias for off-segment scores
NCH = (T + 3) // 4        # AllGather chunks (4 steps each)
NROW = NCH * 2            # fc output rows of 128 positions

_COMPILED = {}


def _build(n_pad, use_b0, use_b1):
    import concourse.bacc as bacc
    import concourse.mybir as mybir
    import concourse.tile as tile
    from concourse.alu_op_type import AluOpType
    from contextlib import ExitStack

    f32 = mybir.dt.float32
    bf16 = mybir.dt.bfloat16
    AF = mybir.ActivationFunctionType
    ADD, MULT = AluOpType.add, AluOpType.mult

    nk = n_pad // 128         # node K-tiles
    nck = (n_pad + 511) // 512  # score column chunks
    nc = bacc.Bacc("TRN2", target_bir_lowering=False, debug=False,
                   num_devices=NCORES)

    D = {}
    def din(name, shape, dt=bf16):
        D[name] = nc.dram_tensor(name, shape, dt, kind="ExternalInput").ap()
        return D[name]

    nfT = din("nfT", [128, 5, n_pad])          # [NF.T; ones-row; 0] blocks
    wcT = din("wcT", [128, 5, 512])            # [W_c.T; b_c; 0] blocks
    wcaT = din("wcaT", [128, 5, 512])    # [(W_c.T@W_a)/2; (b_c@W_a)/2] blocks
    msk = din("msk", [128, n_pad])       # mask rhs block (incl np@b_a fold)
    mi8 = din("mi8", [128, 8])           # mask lhsT block (I8 + ones row)
    i8b = din("i8b", [8, 8])             # identity (bf16)
    embT = din("embT", [128, 4, POS])          # emb.T blocks, cols t*8+b
    wembT = din("wembT", [128, 4, G4])         # W_ih0[:, :512].T blocks scaled
    b0c = din("b0c", [128, G4], f32)     # b0 broadcast (only if used)
    w0T = din("w0T", [128, 8, G4])             # [W_ctx.T; W_hh0.T] scaled
    w1T = din("w1T", [128, 8, G4])             # [W_ih1.T; W_hh1.T] scaled
    b1r = din("b1r", [8, G4])            # b1 rows (only if used)
    gfT = din("gfT", [128, 4, 8])              # 2*graph_features.T blocks
    wfcb = din("wfcb", [128, 4, VPAD])         # W_fc.T/2 shard blocks
    out_d = nc.dram_tensor("out", [NROW, 128, VPAD], bf16,
                           kind="ExternalOutput").ap()

    with tile.TileContext(nc) as tc, ExitStack() as ctx:
        res = ctx.enter_context(tc.tile_pool(name="res", bufs=1))
        dram = ctx.enter_context(tc.tile_pool(name="dram", bufs=1, space="DRAM"))
        drsh = ctx.enter_context(tc.tile_pool(name="drsh", bufs=1, space="DRAM"))

        npT = res.tile([128, 5, n_pad], bf16, tag="npT")   # [NPa.T blocks; mask]
        npB = res.tile([128, nk, 512], bf16, tag="npB")    # NP node-major blocks
        i8bs = res.tile([8, 8], bf16, tag="i8bs")
        msT = res.tile([128, 8], bf16, tag="msT")
        hall = res.tile([128, 4, POS], bf16, tag="hall")
        x0T = res.tile([128, 8, 8], bf16, tag="x0T")       # [ctx.T | H0.T]
        x1T = res.tile([128, 8, 8], bf16, tag="x1T")       # [H0n.T | H1.T]
        c0s = res.tile([8, H], f32, tag="c0s")
        c1s = res.tile([8, H], f32, tag="c1s")
        w0s = res.tile([128, 8, G4], bf16, tag="w0s")
        w1s = res.tile([128, 8, G4], bf16, tag="w1s")
        wfcs = res.tile([128, 4, VPAD], bf16, tag="wfcs")
        hA = [res.tile([128, 4, 256], bf16, tag=f"hA{ch}", name=f"hA{ch}")
              for ch in range(NCH)]
        b1s = res.tile([8, G4], bf16, tag="b1s") if use_b1 else None

        nc.sync.dma_start(i8bs[:], i8b[:])
        nc.sync.dma_start(msT[:], mi8[:])
        nc.sync.dma_start(npT[:, 4, :], msk[:])
        nc.sync.dma_start(x0T[:, 4:8, :], gfT[:])
        nc.sync.dma_start(x1T[:, 4:8, :], gfT[:])
        nc.scalar.dma_start(w0s[:], w0T[:])
        nc.scalar.dma_start(w1s[:], w1T[:])
        nc.scalar.dma_start(wfcs[:], wfcb[:])
        nc.gpsimd.memset(c0s[:], 0.0)
        nc.gpsimd.memset(c1s[:], 0.0)
        if use_b1:
            nc.sync.dma_start(b1s[:], b1r[:])

        eg_dram = dram.tile([POS, G4], bf16)
        ag_ins = [dram.tile([512, 32], bf16, tag=f"agi{i}", name=f"agi{i}")
                  for i in range(NCH)]
        ag_outs = [drsh.tile([NCORES * 512, 32], bf16,
                             addr_space="Shared", tag=f"ago{i}",
                             name=f"ago{i}")
                   for i in range(NCH)]

        # ---------------- phase A: NP.T (scores side), NP, EG0 ----------
        with tc.tile_pool(name="pha", bufs=1, side="right") as pha, \
             tc.tile_pool(name="phap", bufs=1, space="PSUM") as phap:
            nfs = pha.tile([128, 5, n_pad], bf16, tag="nfs")
            wcs = pha.tile([128, 5, 512], bf16, tag="wcs")
            was = pha.tile([128, 5, 512], bf16, tag="was")
            nc.sync.dma_start(nfs[:], nfT[:])
            nc.sync.dma_start(wcs[:], wcT[:])
            nc.sync.dma_start(was[:], wcaT[:])

            # NPa.T chunk mt = sum_kt was[:,kt,mt-chunk].T @ nfs[:,kt,:]
            for mt in range(4):
                p = phap.tile([128, n_pad], f32, tag="pa")
                for kt in range(5):
                    lt = was[:, kt, mt * 128:(mt + 1) * 128]
                    for c0 in range(0, n_pad, 512):
                        cw = min(512, n_pad - c0)
                        nc.tensor.matmul(
                            p[:, c0:c0 + cw], lt,
                            nfs[:, kt, c0:c0 + cw],
                            start=(kt == 0), stop=(kt == 4))
                nc.scalar.copy(npT[:, mt, :], p[:])

            # NP block j = sum_kt nfs[:,kt,j-chunk].T @ wcs[:,kt,:]
            for j in range(nk):
                p = phap.tile([128, 512], f32, tag="pb")
                for kt in range(5):
                    nc.tensor.matmul(
                        p[:], nfs[:, kt, j * 128:(j + 1) * 128],
                        wcs[:, kt, :], start=(kt == 0), stop=(kt == 4))
                nc.scalar.copy(npB[:, j, :], p[:])

            # EG0 [POS, 2048] = embT.T @ wembT (+ b0)
            ems = pha.tile([128, 4, POS], bf16, tag="ems")
            nc.sync.dma_start(ems[:], embT[:])
            if use_b0:
                b0s = pha.tile([128, G4], f32, tag="b0s")
                nc.sync.dma_start(b0s[:], b0c[:])
            for mc in range(0, POS, 128):
                mw = min(128, POS - mc)
                p = phap.tile([128, G4], f32, tag="pc")
                for c0 in range(0, G4, 512):
                    wes = pha.tile([128, 4, 512], bf16, tag="wes", bufs=2)
                    nc.sync.dma_start(wes[:], wembT[:, :, c0:c0 + 512])
                    for kt in range(4):
                        nc.tensor.matmul(
                            p[:mw, c0:c0 + 512],
                            ems[:, kt, mc:mc + mw],
                            wes[:, kt, :],
                            start=(kt == 0), stop=(kt == 3))
                for h0_ in (0, 1024):
                    eo = pha.tile([128, 1024], bf16, tag="eo")
                    if use_b0:
                        nc.vector.tensor_add(eo[:mw, :], p[:mw, h0_:h0_ + 1024],
                                             b0s[:mw, h0_:h0_ + 1024])
                    else:
                        nc.scalar.copy(eo[:mw, :], p[:mw, h0_:h0_ + 1024])
                    nc.sync.dma_start(eg_dram[mc:mc + mw, h0_:h0_ + 1024],
                                      eo[:mw, :])

        # ---------------- recurrence + interleaved fc ----------------
        fc_row = [0]      # next output row (ch*2+pc), 0..NROW-1
        fc_vc = [0]       # next vocab chunk within row, 0..7
        fc_cur = [None]   # current fco tile

        with tc.tile_pool(name="stepp", bufs=1) as stepp, \
             tc.tile_pool(name="egp", bufs=2) as egp, \
             tc.tile_pool(name="fco", bufs=2) as fco, \
             tc.tile_pool(name="gp", bufs=2, space="PSUM") as gp, \
             tc.tile_pool(name="scp", bufs=2, space="PSUM") as scp, \
             tc.tile_pool(name="sml", bufs=1, space="PSUM") as sml, \
             tc.tile_pool(name="fcp", bufs=3, space="PSUM") as fcp:

            def fc_unit():
                """One (row, vc) fc unit: 4 matmuls + copy; DMA on row end."""
                row, vc = fc_row[0], fc_vc[0]
                ch, pc = divmod(row, 2)
                if vc == 0:
                    fc_cur[0] = fco.tile([128, VPAD], bf16, tag="fcr",
                                         name=f"fcr{row}")
                p = fcp.tile([128, 512], f32, tag="fc")
                for kt in range(4):
                    nc.tensor.matmul(p[:], hA[ch][:, kt, pc * 128:(pc + 1) * 128],
                                     wfcs[:, kt, vc * 512:(vc + 1) * 512],
                                     start=(kt == 0), stop=(kt == 3))
                nc.scalar.copy(fc_cur[0][:, vc * 512:(vc + 1) * 512], p[:])
                fc_vc[0] += 1
                if fc_vc[0] == 8:
                    nc.sync.dma_start(out_d[row], fc_cur[0][:])
                    fc_row[0] += 1
                    fc_vc[0] = 0

            def cell(Tg, cS, dsts, hall_slice=None):
                """LSTM cell from tanh'd gates Tg [8,2048] (t_i|t_f|t_g|t_o
                with i,f,o pre-halved); updates cS (=2c) in place, writes
                the transposed doubled hidden state into dsts."""
                u = stepp.tile([8, 512], f32, tag="u")
                nc.vector.scalar_tensor_tensor(
                    u[:], Tg[:, 512:1024], 1.0, cS[:], ADD, MULT)
                v = stepp.tile([8, 512], f32, tag="v")
                nc.vector.scalar_tensor_tensor(
                    v[:], Tg[:, 0:512], 1.0, Tg[:, 1024:1536], ADD, MULT)
                nc.vector.scalar_tensor_tensor(
                    cS[:], u[:], 0.5, v[:], MULT, ADD)
                tch = stepp.tile([8, 512], f32, tag="tch")
                nc.scalar.activation(tch[:], cS[:], AF.Tanh, scale=0.5)
                hn = stepp.tile([8, 512], bf16, tag="hn")
                nc.vector.scalar_tensor_tensor(
                    hn[:], Tg[:, 1536:2048], 1.0, tch[:], ADD, MULT)
                tp = sml.tile([128, 96], bf16, tag="tp")
                for j in range(4):
                    nc.tensor.transpose(tp[:, j * 8:(j + 1) * 8],
                                        hn[:, j * 128:(j + 1) * 128], i8bs[:])
                tpv = tp[:, 0:32].rearrange("p (a b) -> p a b", a=4)
                for dst in dsts:
                    nc.vector.tensor_copy(dst, tpv)
                if hall_slice is not None:
                    nc.vector.tensor_copy(hall_slice, tpv)

            for t in range(T):
                eg = egp.tile([8, G4], bf16, tag="eg")
                nc.sync.dma_start(eg[:], eg_dram[t * 8:(t + 1) * 8, :])

                # scores S.T [8, n_pad] = H1/2 @ NPa.T + mask, in 512-chunks
                Et = stepp.tile([8, n_pad], bf16, tag="Et")
                dp = stepp.tile([8, 4], f32, tag="dp")
                for c in range(nck):
                    c0 = c * 512
                    cw = min(512, n_pad - c0)
                    sc = scp.tile([8, 512], f32, tag="sc")
                    for kt in (4, 0, 1, 2, 3):
                        lt = msT[:] if kt == 4 else x1T[:, 4 + kt, :]
                        nc.tensor.matmul(sc[:, 0:cw], lt,
                                         npT[:, kt, c0:c0 + cw],
                                         start=(kt == 4), stop=(kt == 3))
                    nc.scalar.activation(Et[:, c0:c0 + cw], sc[:, 0:cw],
                                         AF.Exp, accum_out=dp[:, c:c + 1])
                den = stepp.tile([8, 1], f32, tag="den")
                if nck == 1:
                    den = dp[:, 0:1]
                else:
                    nc.vector.tensor_add(den[:], dp[:, 0:1], dp[:, 1:2])
                    for c in range(2, nck):
                        nc.vector.tensor_add(den[:], den[:], dp[:, c:c + 1])
                r8 = stepp.tile([8, 1], f32, tag="r8")
                nc.vector.reciprocal(r8[:], den[:])

                # E.T via PE transposes
                etP = sml.tile([128, 96], bf16, tag="tp")
                for j in range(nk):
                    nc.tensor.transpose(etP[:, j * 8:(j + 1) * 8],
                                        Et[:, j * 128:(j + 1) * 128], i8bs[:])
                etT = stepp.tile([128, nk, 8], bf16, tag="etT")
                nc.vector.tensor_copy(
                    etT[:], etP[:, 0:nk * 8].rearrange("p (a b) -> p a b", a=nk))

                # ctx [8, 512] = E @ NP, scaled by 1/den on copy-out
                ctxP = scp.tile([8, 512], f32, tag="sc")
                for j in range(nk):
                    nc.tensor.matmul(ctxP[:], etT[:, j, :], npB[:, j, :],
                                     start=(j == 0), stop=(j == nk - 1))
                ctxS = stepp.tile([8, 512], bf16, tag="ctxS")
                nc.scalar.activation(ctxS[:], ctxP[:], AF.Copy, scale=r8[:])

                # ctx.T -> x0T[:, 0:4, :]
                ctP = sml.tile([128, 96], bf16, tag="tp")
                for j in range(4):
                    nc.tensor.transpose(ctP[:, j * 8:(j + 1) * 8],
                                        ctxS[:, j * 128:(j + 1) * 128], i8bs[:])
                nc.vector.tensor_copy(
                    x0T[:, 0:4, :],
                    ctP[:, 0:32].rearrange("p (a b) -> p a b", a=4))

                # gates0 in [8,512] quarters: sum_kt x0T.T @ w0 + EG0[t]
                Tg0 = stepp.tile([8, G4], f32, tag="Tg0")
                for q in range(4):
                    qs = q * 512
                    g = gp.tile([8, 512], f32, tag="g")
                    for kt in range(8):
                        nc.tensor.matmul(g[:], x0T[:, kt, :],
                                         w0s[:, kt, qs:qs + 512],
                                         start=(kt == 0), stop=False)
                    nc.tensor.matmul(g[:], i8bs[:], eg[:, qs:qs + 512],
                                     start=False, stop=True)
                    nc.scalar.activation(Tg0[:, qs:qs + 512], g[:], AF.Tanh)
                cell(Tg0, c0s, [x1T[:, 0:4, :], x0T[:, 4:8, :]])

                # gates1: h1-parts of q0/q1 early (overlap cell0), then close
                Tg1 = stepp.tile([8, G4], f32, tag="Tg1")
                g1q = [None] * 4
                def g1_open(q):
                    g = gp.tile([8, 512], f32, tag="g")
                    g1q[q] = g
                    for kt in range(4, 8):
                        nc.tensor.matmul(g[:], x1T[:, kt, :],
                                         w1s[:, kt, q * 512:q * 512 + 512],
                                         start=(kt == 4), stop=False)
                def g1_close(q):
                    g = g1q[q]
                    qs = q * 512
                    for kt in range(4):
                        nc.tensor.matmul(g[:], x1T[:, kt, :],
                                         w1s[:, kt, qs:qs + 512],
                                         start=False,
                                         stop=(kt == 3 and not use_b1))
                    if use_b1:
                        nc.tensor.matmul(g[:], i8bs[:], b1s[:, qs:qs + 512],
                                         start=False, stop=True)
                    nc.scalar.activation(Tg1[:, qs:qs + 512], g[:], AF.Tanh)
                g1_open(0)
                g1_open(1)
                g1_close(0)
                g1_close(1)
                g1_open(2)
                g1_close(2)
                g1_open(3)
                g1_close(3)

                # interleave fc work into the cell1 window
                avail_rows = 0 if t < 6 else min(NROW, 2 * ((t - 6) // 4 + 1))
                budget = 2
                while budget > 0 and fc_row[0] < avail_rows:
                    fc_unit()
                    budget -= 1

                cell(Tg1, c1s, [x1T[:, 4:8, :]],
                     hall_slice=hall[:, :, t * 8:(t + 1) * 8])

                if t % 4 == 3:
                    ch = t // 4
                    agi = ag_ins[ch]
                    nc.sync.dma_start(
                        agi[:].rearrange("(a p) n -> p a n", p=128),
                        hall[:, :, ch * 32:(ch + 1) * 32])
                    nc.gpsimd.collective_compute(
                        "AllGather", mybir.AluOpType.bypass,
                        replica_groups=[list(range(NCORES))],
                        ins=[agi.opt()], outs=[ag_outs[ch].opt()])
                    for c in range(NCORES):
                        nc.scalar.dma_start(
                            hA[ch][:, :, c * 32:(c + 1) * 32],
                            ag_outs[ch][c * 512:(c + 1) * 512].rearrange(
                                "(a p) n -> p a n", p=128))

            # ---------------- fc tail ----------------
            while fc_row[0] < NROW:
                fc_unit()

    nc.compile()
    return nc


def _prep(inputs, n_pad):
    import ml_dtypes
    bf = ml_dtypes.bfloat16
    gf = np.ascontiguousarray(np.asarray(inputs["graph_features"], np.float32))
    nf = np.ascontiguousarray(np.asarray(inputs["node_features"], np.float32))
    emb = np.asarray(inputs["embedding"], np.float32)
    W_a = np.asarray(inputs["W_a"], np.float32)
    b_a = np.asarray(inputs["b_a"], np.float32)
    W_c = np.asarray(inputs["W_c"], np.float32)
    b_c = np.asarray(inputs["b_c"], np.float32)
    W_ih0 = np.asarray(inputs["W_ih0"], np.float32)
    W_hh0 = np.asarray(inputs["W_hh0"], np.float32)
    b0 = np.asarray(inputs["b_ih0"], np.float32) + np.asarray(inputs["b_hh0"], np.float32)
    W_ih1 = np.asarray(inputs["W_ih1"], np.float32)
    W_hh1 = np.asarray(inputs["W_hh1"], np.float32)
    b1 = np.asarray(inputs["b_ih1"], np.float32) + np.asarray(inputs["b_hh1"], np.float32)
    W_fc = np.asarray(inputs["W_fc"], np.float32)
    bidx = np.asarray(inputs["batch_idx"]).astype(np.int64)
    caps = np.asarray(inputs["captions"]).astype(np.int64)

    # gate scale: i,f,o gates halved (sigmoid-via-tanh); g full.
    gsc = np.ones((G4,), np.float32) * 0.5
    gsc[2 * H:3 * H] = 1.0        # g gate (order i,f,g,o)
    # h-doubling: consumers of h scale by 0.5
    w0 = np.concatenate([W_ih0[:, 512:].T * gsc[None, :],
                         W_hh0.T * (0.5 * gsc)[None, :]], 0)
    w1 = np.concatenate([W_ih1.T * (0.5 * gsc)[None, :],
                         W_hh1.T * (0.5 * gsc)[None, :]], 0)
    wemb = W_ih0[:, :512].T * gsc[None, :]
    b0s = b0 * gsc
    b1s = b1 * gsc

    def blocks(a):
        K, N = a.shape
        return np.ascontiguousarray(a.reshape(K // 128, 128, N).transpose(1, 0, 2))

    wcT_full = np.zeros((640, 512), np.float32)
    wcT_full[:512] = W_c.T
    wcT_full[512] = b_c
    wca_full = np.zeros((640, 512), np.float32)
    wca_full[:512] = 0.5 * (W_c.T @ W_a)
    wca_full[512] = 0.5 * (b_c @ W_a)
    i8 = np.eye(8, dtype=np.float32)
    mi8 = np.zeros((128, 8), np.float32)
    mi8[:8, :8] = np.eye(8)
    mi8[8, :] = 1.0
    b0c = np.tile(b0s[None, :], (128, 1)).astype(np.float32)
    b1r = np.tile(b1s[None, :], (8, 1))
    use_b0 = bool(np.any(b0 != 0))
    use_b1 = bool(np.any(b1 != 0))
    sb_ba = (nf @ W_c.T + b_c) @ b_a      # per-node b_a fold for scores

    maps = []
    for k in range(NCORES):
        sel = (bidx >= k * BL) & (bidx < (k + 1) * BL)
        nodes = np.nonzero(sel)[0]
        cnt = len(nodes)
        nfT_full = np.zeros((640, n_pad), np.float32)
        nfT_full[:512, :cnt] = nf[nodes].T
        nfT_full[512, :cnt] = 1.0
        lb = bidx[nodes] - k * BL
        msk = np.zeros((128, n_pad), np.float32)
        msk[8, :] = NEG
        msk[8, :cnt] += sb_ba[nodes]
        msk[lb, np.arange(cnt)] = -NEG
        e = emb[caps[k * BL:(k + 1) * BL]]             # [8, T, E]
        embT_full = np.ascontiguousarray(e.transpose(2, 1, 0).reshape(E, POS))
        wfc = np.zeros((VPAD, H), np.float32)
        wfc[:VSH] = 0.5 * W_fc[k * VSH:(k + 1) * VSH]
        wfcb = blocks(np.ascontiguousarray(wfc.T))     # [128, 4, VPAD]
        m = {
            "nfT": blocks(nfT_full).astype(bf),
            "wcT": blocks(wcT_full).astype(bf),
            "wcaT": blocks(wca_full).astype(bf),
            "msk": msk.astype(bf), "mi8": mi8.astype(bf),
            "i8b": i8.astype(bf),
            "embT": blocks(embT_full).astype(bf),
            "wembT": blocks(wemb).astype(bf), "b0c": b0c,
            "w0T": blocks(w0).astype(bf), "w1T": blocks(w1).astype(bf),
            "b1r": b1r.astype(bf),
            "gfT": blocks(np.ascontiguousarray(
                2.0 * gf[k * BL:(k + 1) * BL].T)).astype(bf),
            "wfcb": wfcb.astype(bf),
        }
        maps.append(m)
    return maps, use_b0, use_b1


def kernel(**inputs) -> np.ndarray:
    from concourse.bass_utils import run_bass_kernel_spmd

    bidx = np.asarray(inputs["batch_idx"]).astype(np.int64)
    counts = np.bincount(bidx // BL, minlength=NCORES)
    n_pad = max(256, int(math.ceil(counts.max() / 128.0)) * 128)
    maps, use_b0, use_b1 = _prep(inputs, n_pad)
    key = (n_pad, use_b0, use_b1)
    if key not in _COMPILED:
        _COMPILED[key] = _build(n_pad, use_b0, use_b1)
    res = run_bass_kernel_spmd(_COMPILED[key], maps,
                               core_ids=list(range(NCORES)))
    b_fc = np.asarray(inputs["b_fc"], np.float32)
    out = np.empty((B, T, V), np.float32)
    for k in range(NCORES):
        o = np.asarray(res.results[k]["out"]).astype(np.float32)
        # [ch, pc, c4, dt, b, v] -> [pc, c4, b, ch, dt, v]
        o = o.reshape(NCH, 2, 4, 4, 8, VPAD).transpose(1, 2, 4, 0, 3, 5)
        out[:, :, k * VSH:(k + 1) * VSH] = o.reshape(B, T, VPAD)[:, :, :VSH]
    if np.any(b_fc != 0):
        out += b_fc[None, None, :]
    return out



# revision 25
# speedup vs baseline: 1.6442x; 1.6442x over previous
"""AttentionLSTMDecoder Trainium2 kernel (8-core SPMD), v3.

Sharding: data-parallel over batch B=64 -> 8 graphs/core for the
recurrent part (attention over that core's node segment + 2-layer LSTM),
AllGather of h1 trajectories in 4-step chunks (bf16), vocab-sharded fc
(each core computes a 4096-wide padded vocab slice for all positions).

v3 changes vs v2:
- ALL recurrence matmuls use 4x PE column-tiling: the four quarters of
  every M=8 matmul run concurrently in independent 128x32 PE tiles
  (outputs at PSUM partitions 32j), ~4x faster streaming.
- "banded" data layout: band j (partitions 32j..32j+8) holds batch rows
  for the j-th 128-wide h-slice. All ACT/DVE elementwise work becomes
  [128, 128..512]-shaped (one instruction instead of four, 4x less
  free-dim per lane).
- DVE 32x32 block transposes (vector.transpose) replace PE transposes;
  a fixed h-permutation PERM(p,c) = 128*(p//32)+32*c+(p%32), applied
  host-side to all weight matrices, makes the block-transposed banded
  tensors directly usable as matmul stationaries.
- segment-softmax denominator via tiny per-band column-sum matmuls.
- fc also issued as 4 concurrent 32-row band matmuls (same tile mode,
  no PE mode switches inside the loop).
"""

import math

import numpy as np

B, T, H, E, V, NTOT = 64, 20, 512, 512, 32000, 8192
NCORES = 8
BL = B // NCORES          # 8 graphs per core
POS = T * BL              # 160 positions per core
VSH = V // NCORES         # 4000 vocab rows per core
VPAD = 4096               # padded vocab shard
G4 = 4 * H                # 2048 gate width
NEG = -40.0               # mask bias for off-segment scores
NCH = (T + 3) // 4        # AllGather chunks (4 steps each)
NROW = NCH * 2            # fc output rows of 128 positions

_COMPILED = {}


def _build(n_pad, use_b0, use_b1):
    import concourse.bacc as bacc
    import concourse.mybir as mybir
    import concourse.tile as tile
    from concourse.alu_op_type import AluOpType
    from contextlib import ExitStack

    f32 = mybir.dt.float32
    bf16 = mybir.dt.bfloat16
    AF = mybir.ActivationFunctionType
    ADD, MULT = AluOpType.add, AluOpType.mult

    NB = n_pad // 4           # nodes per band
    nkc = NB // 32            # 32-node blocks per band = ctx K-chunks
    nck = (n_pad + 511) // 512
    nc = bacc.Bacc("TRN2", target_bir_lowering=False, debug=False,
                   num_devices=NCORES)

    D = {}
    def din(name, shape, dt=bf16):
        D[name] = nc.dram_tensor(name, shape, dt, kind="ExternalInput").ap()
        return D[name]

    npTi = din("npTi", [128, 5, n_pad])  # NPa.T perm chunks + mask block
    npBi = din("npBi", [128, nkc, 512])  # NP rows, chunk-node-major
    egd = din("egd", [POS, G4])          # emb@W_ih0[:, :512] (+b0), banded
    mi32 = din("mi32", [128, 32])        # mask lhsT: I8 + ones row, 32 cols
    i8p = din("i8p", [128, 8])           # identity rows 0-7, zero below
    w0T = din("w0T", [128, 8, G4])             # [W_ctx.T; W_hh0.T] perm/banded
    w1T = din("w1T", [128, 8, G4])             # [W_ih1.T; W_hh1.T] perm/banded
    b1r = din("b1r", [8, G4])            # b1 rows banded cols (only if used)
    gfT = din("gfT", [128, 4, 8])              # 2*graph_features.T perm blocks
    wfcb = din("wfcb", [128, 4, VPAD])         # W_fc.T/2 shard, perm rows
    bsum = din("bsum", [128, 128], f32)  # band-sum matrix (k%32==m%32<8)
    out_d = nc.dram_tensor("out", [NROW, 128, VPAD], bf16,
                           kind="ExternalOutput").ap()

    with tile.TileContext(nc) as tc, ExitStack() as ctx:
        res = ctx.enter_context(tc.tile_pool(name="res", bufs=1))
        dram = ctx.enter_context(tc.tile_pool(name="dram", bufs=1, space="DRAM"))
        drsh = ctx.enter_context(tc.tile_pool(name="drsh", bufs=1, space="DRAM"))

        npT = res.tile([128, 5, n_pad], bf16, tag="npT")   # [NPa.T blocks; mask]
        npB = res.tile([128, nkc, 512], bf16, tag="npB")   # NP chunk-node-major
        i8s = res.tile([128, 8], bf16, tag="i8s")
        msT = res.tile([128, 32], bf16, tag="msT")
        bss = res.tile([128, 128], f32, tag="bss")
        hall = res.tile([128, 4, POS], bf16, tag="hall")
        hT0 = [res.tile([128, 128], bf16, tag=f"hT0{i}", name=f"hT0{i}")
               for i in range(2)]
        hT1 = [res.tile([128, 128], bf16, tag=f"hT1{i}", name=f"hT1{i}")
               for i in range(2)]
        c0s = res.tile([128, 128], f32, tag="c0s")
        c1s = res.tile([128, 128], f32, tag="c1s")
        w0s = res.tile([128, 8, G4], bf16, tag="w0s")
        w1s = res.tile([128, 8, G4], bf16, tag="w1s")
        wfcs = res.tile([128, 4, VPAD], bf16, tag="wfcs")
        egs = [res.tile([128, G4], bf16, tag=f"egs{i}", name=f"egs{i}")
               for i in range(2)]
        hA = [res.tile([128, 4, 256], bf16, tag=f"hA{ch}", name=f"hA{ch}")
              for ch in range(NCH)]
        b1s = res.tile([128, G4], bf16, tag="b1s") if use_b1 else None

        ag_ins = [dram.tile([512, 32], bf16, tag=f"agi{i}", name=f"agi{i}")
                  for i in range(NCH)]
        ag_outs = [drsh.tile([NCORES * 512, 32], bf16,
                             addr_space="Shared", tag=f"ago{i}",
                             name=f"ago{i}")
                   for i in range(NCH)]

        # ---------------- input DMAs (phase A precomputed on host) -----
        engs = [nc.sync, nc.scalar, nc.gpsimd]
        nc.gpsimd.memset(c0s[:], 0.0)
        nc.gpsimd.memset(c1s[:], 0.0)
        nc.gpsimd.memset(egs[0][:], 0.0)
        nc.gpsimd.memset(egs[1][:], 0.0)
        if use_b1:
            nc.gpsimd.memset(b1s[:], 0.0)
            nc.sync.dma_start(b1s[0:8, :], b1r[:])
        nc.gpsimd.dma_start(
            hT0[0][:].rearrange("p (c n) -> p c n", c=4)[:, :, 0:8], gfT[:])
        nc.gpsimd.dma_start(
            hT1[0][:].rearrange("p (c n) -> p c n", c=4)[:, :, 0:8], gfT[:])
        nc.sync.dma_start(i8s[:], i8p[:])
        nc.scalar.dma_start(msT[:], mi32[:])
        nc.sync.dma_start(bss[:], bsum[:])
        n = 0
        for i in range(5):
            engs[n % 3].dma_start(npT[:, i, :], npTi[:, i, :]); n += 1
        for i in (4, 5, 6, 7):
            engs[n % 3].dma_start(w0s[:, i, :], w0T[:, i, :]); n += 1
        for i in range(nkc):
            engs[n % 3].dma_start(npB[:, i, :], npBi[:, i, :]); n += 1
        for i in (0, 1, 2, 3):
            engs[n % 3].dma_start(w0s[:, i, :], w0T[:, i, :]); n += 1
        for i in (4, 5, 6, 7):
            engs[n % 3].dma_start(w1s[:, i, :], w1T[:, i, :]); n += 1
        for i in (0, 1, 2, 3):
            engs[n % 3].dma_start(w1s[:, i, :], w1T[:, i, :]); n += 1
        for i in range(4):
            engs[n % 3].dma_start(wfcs[:, i, :], wfcb[:, i, :]); n += 1

        # ---------------- recurrence + interleaved fc ----------------
        fc_row = [0]      # next output row (ch*2+pc), 0..NROW-1
        fc_vc = [0]       # next vocab chunk within row, 0..7
        fc_cur = [None]   # current fco tile

        with tc.tile_pool(name="stepp", bufs=1) as stepp, \
             tc.tile_pool(name="fco", bufs=2) as fco, \
             tc.tile_pool(name="gp0", bufs=2, space="PSUM") as gp0, \
             tc.tile_pool(name="gp1", bufs=2, space="PSUM") as gp1, \
             tc.tile_pool(name="scp", bufs=1, space="PSUM") as scp, \
             tc.tile_pool(name="ctp", bufs=1, space="PSUM") as ctp, \
             tc.tile_pool(name="fcp", bufs=2, space="PSUM") as fcp:

            def fc_unit():
                """One (row, vc) fc unit: 16 band matmuls + copy; DMA on
                row end."""
                row, vc = fc_row[0], fc_vc[0]
                ch, pc = divmod(row, 2)
                if vc == 0:
                    fc_cur[0] = fco.tile([128, VPAD], bf16, tag="fcr",
                                         name=f"fcr{row}")
                p = fcp.tile([128, 512], f32, tag="fc")
                for kt in range(4):
                    for j in range(4):
                        nc.tensor.matmul(
                            p[32 * j:32 * j + 32, :],
                            hA[ch][:, kt, pc * 128 + 32 * j:pc * 128 + 32 * j + 32],
                            wfcs[:, kt, vc * 512:(vc + 1) * 512],
                            start=(kt == 0), stop=(kt == 3),
                            tile_position=(0, 32 * j))
                if vc % 2 == 0:
                    nc.scalar.copy(fc_cur[0][:, vc * 512:(vc + 1) * 512], p[:])
                else:
                    nc.vector.tensor_copy(
                        fc_cur[0][:, vc * 512:(vc + 1) * 512], p[:])
                fc_vc[0] += 1
                if fc_vc[0] == 4:
                    nc.sync.dma_start(out_d[row][:, 0:2048],
                                      fc_cur[0][:, 0:2048])
                if fc_vc[0] == 8:
                    nc.sync.dma_start(out_d[row][:, 2048:VPAD],
                                      fc_cur[0][:, 2048:VPAD])
                    fc_row[0] += 1
                    fc_vc[0] = 0

            def fc_budget(t, budget):
                if t < 10:
                    avail_rows = 0
                else:
                    avail_rows = min(NROW, 2 * ((t - 8) // 4 + 1))
                while budget > 0 and fc_row[0] < avail_rows:
                    fc_unit()
                    budget -= 1

            def cell(Tg, cS, hTn):
                """Banded LSTM cell from tanh'd gates Tg [128,512]
                (t_i|t_f|t_g|t_o per 128-col block, i,f,o pre-halved);
                updates cS (=2c) in place, writes block-transposed doubled
                hidden state into hTn [128,128]."""
                v = stepp.tile([128, 128], f32, tag="v")
                nc.vector.scalar_tensor_tensor(
                    v[:], Tg[:, 0:128], 1.0, Tg[:, 256:384], ADD, MULT)
                u = stepp.tile([128, 128], f32, tag="u")
                nc.vector.scalar_tensor_tensor(
                    u[:], Tg[:, 128:256], 1.0, cS[:], ADD, MULT)
                nc.vector.scalar_tensor_tensor(
                    cS[:], u[:], 0.5, v[:], MULT, ADD)
                tch = stepp.tile([128, 128], f32, tag="tch")
                nc.scalar.activation(tch[:], cS[:], AF.Tanh, scale=0.5)
                hn = stepp.tile([128, 128], bf16, tag="hn")
                nc.vector.scalar_tensor_tensor(
                    hn[:], Tg[:, 384:512], 1.0, tch[:], ADD, MULT)
                nc.vector.transpose(hTn[:, 0:64], hn[:, 0:64])
                nc.vector.transpose(hTn[:, 64:128], hn[:, 64:128])

            nc.sync.dma_start(egs[0][0:8, :], egd[0:8, :])
            for t in range(T):
                h0c = hT0[t % 2][:].rearrange("p (c n) -> p c n", c=4)
                h1c = hT1[t % 2][:].rearrange("p (c n) -> p c n", c=4)
                h0n = hT0[(t + 1) % 2]
                h1n = hT1[(t + 1) % 2]
                eg = egs[t % 2]
                if t + 1 < T:
                    nc.sync.dma_start(egs[(t + 1) % 2][0:8, :],
                                      egd[(t + 1) * 8:(t + 2) * 8, :])

                # g0 pre-run: eg bias + h0-part (no dep on this step's ctx)
                g0P = gp0.tile([128, 512], f32, tag="g0")
                for j in range(4):
                    nc.tensor.matmul(g0P[32 * j:32 * j + 8, :], i8s[:],
                                     eg[:, j * 512:(j + 1) * 512],
                                     start=True, stop=False,
                                     tile_position=(0, 32 * j))
                for c in range(4):
                    for j in range(4):
                        nc.tensor.matmul(g0P[32 * j:32 * j + 8, :],
                                         h0c[:, c, 0:8],
                                         w0s[:, 4 + c, j * 512:(j + 1) * 512],
                                         start=False, stop=False,
                                         tile_position=(0, 32 * j))

                # scores S banded [32j+b, NB] = (mask + H1/2 @ NPa.T)
                scP = scp.tile([128, NB + 32], f32, tag="sc")
                for j in range(4):
                    nc.tensor.matmul(scP[32 * j:32 * j + 32, 0:NB], msT[:],
                                     npT[:, 4, j * NB:(j + 1) * NB],
                                     start=True, stop=False,
                                     tile_position=(0, 32 * j))
                for c in range(4):
                    for j in range(4):
                        nc.tensor.matmul(scP[32 * j:32 * j + 8, 0:NB],
                                         h1c[:, c, 0:8],
                                         npT[:, c, j * NB:(j + 1) * NB],
                                         start=False, stop=(c == 3),
                                         tile_position=(0, 32 * j))
                fc_budget(t, 3)

                NBa = (nkc // 2) * 32
                Eta = stepp.tile([128, NBa], bf16, tag="Eta", bufs=2)
                Etb = stepp.tile([128, NB - NBa], bf16, tag="Etb", bufs=2)
                dp = stepp.tile([128, 2], f32, tag="dp")
                nc.scalar.activation(Eta[:], scP[:, 0:NBa], AF.Exp,
                                     accum_out=dp[:, 0:1])
                nc.scalar.activation(Etb[:], scP[:, NBa:NB], AF.Exp,
                                     accum_out=dp[:, 1:2])
                # e.T via DVE 32x32 block transposes (pipelined halves)
                eTa = stepp.tile([128, NBa], bf16, tag="eTa", bufs=2)
                eTb = stepp.tile([128, NB - NBa], bf16, tag="eTb", bufs=2)
                nc.vector.transpose(eTa[:], Eta[:])
                nc.vector.transpose(eTb[:], Etb[:])
                # den = band-sum of dp halves, replicated; recip
                for j in range(4):
                    nc.tensor.matmul(scP[32 * j:32 * j + 32, NB:NB + 1],
                                     bss[:, 32 * j:32 * j + 32], dp[:, 0:1],
                                     start=True, stop=False,
                                     tile_position=(0, 32 * j))
                for j in range(4):
                    nc.tensor.matmul(scP[32 * j:32 * j + 32, NB:NB + 1],
                                     bss[:, 32 * j:32 * j + 32], dp[:, 1:2],
                                     start=False, stop=True,
                                     tile_position=(0, 32 * j))
                r = stepp.tile([128, 1], f32, tag="r")
                nc.vector.reciprocal(r[:], scP[:, NB:NB + 1])

                eTav = eTa[:].rearrange("p (k n) -> p k n", k=nkc // 2)
                eTbv = eTb[:].rearrange("p (k n) -> p k n", k=nkc - nkc // 2)
                ctxP = ctp.tile([128, 128], f32, tag="cx")
                for kc in range(nkc):
                    ev = (eTav[:, kc, 0:8] if kc < nkc // 2
                          else eTbv[:, kc - nkc // 2, 0:8])
                    for j in range(4):
                        nc.tensor.matmul(ctxP[32 * j:32 * j + 8, :],
                                         ev,
                                         npB[:, kc, j * 128:(j + 1) * 128],
                                         start=(kc == 0), stop=(kc == nkc - 1),
                                         tile_position=(0, 32 * j))
                ctxS = stepp.tile([128, 128], bf16, tag="ctxS")
                nc.vector.tensor_scalar_mul(ctxS[:], ctxP[:], r[:])
                ctxT = stepp.tile([128, 128], bf16, tag="ctxT", bufs=2)
                nc.vector.transpose(ctxT[:, 0:64], ctxS[:, 0:64])
                nc.vector.transpose(ctxT[:, 64:128], ctxS[:, 64:128])
                ctv = ctxT[:].rearrange("p (c n) -> p c n", c=4)

                # close gates0 with ctx-part; tanh evac; cell0
                for c in range(4):
                    for j in range(4):
                        nc.tensor.matmul(g0P[32 * j:32 * j + 8, :],
                                         ctv[:, c, 0:8],
                                         w0s[:, c, j * 512:(j + 1) * 512],
                                         start=False, stop=(c == 3),
                                         tile_position=(0, 32 * j))
                Tg0 = stepp.tile([128, 512], f32, tag="Tg0")
                nc.scalar.activation(Tg0[:, 0:384], g0P[:, 0:384], AF.Tanh)
                nc.scalar.activation(Tg0[:, 384:512], g0P[:, 384:512], AF.Tanh)

                # g1 pre-run: h1-part (+b1), overlaps cell0
                g1P = gp1.tile([128, 512], f32, tag="g1")
                if use_b1:
                    for j in range(4):
                        nc.tensor.matmul(g1P[32 * j:32 * j + 8, :], i8s[:],
                                         b1s[:, j * 512:(j + 1) * 512],
                                         start=True, stop=False,
                                         tile_position=(0, 32 * j))
                for c in range(4):
                    for j in range(4):
                        nc.tensor.matmul(g1P[32 * j:32 * j + 8, :],
                                         h1c[:, c, 0:8],
                                         w1s[:, 4 + c, j * 512:(j + 1) * 512],
                                         start=(c == 0 and not use_b1),
                                         stop=False,
                                         tile_position=(0, 32 * j))

                cell(Tg0, c0s, h0n)
                h0nv = h0n[:].rearrange("p (c n) -> p c n", c=4)

                fc_budget(t, 1)

                # close gates1 with h0n-part; tanh evac; cell1
                for c in range(4):
                    for j in range(4):
                        nc.tensor.matmul(g1P[32 * j:32 * j + 8, :],
                                         h0nv[:, c, 0:8],
                                         w1s[:, c, j * 512:(j + 1) * 512],
                                         start=False, stop=(c == 3),
                                         tile_position=(0, 32 * j))
                Tg1 = stepp.tile([128, 512], f32, tag="Tg1")
                nc.scalar.activation(Tg1[:, 0:384], g1P[:, 0:384], AF.Tanh)
                nc.scalar.activation(Tg1[:, 384:512], g1P[:, 384:512], AF.Tanh)

                fc_budget(t, 2)

                cell(Tg1, c1s, h1n)
                nc.vector.tensor_copy(
                    hall[:, :, t * 8:(t + 1) * 8],
                    h1n[:].rearrange("p (c n) -> p c n", c=4)[:, :, 0:8])

                if t % 4 == 3:
                    ch = t // 4
                    agi = ag_ins[ch]
                    nc.gpsimd.dma_start(
                        agi[:].rearrange("(a p) n -> p a n", p=128),
                        hall[:, :, ch * 32:(ch + 1) * 32])
                    nc.gpsimd.collective_compute(
                        "AllGather", mybir.AluOpType.bypass,
                        replica_groups=[list(range(NCORES))],
                        ins=[agi.opt()], outs=[ag_outs[ch].opt()])
                    if ch < NCH - 1:
                        for c in range(NCORES):
                            nc.gpsimd.dma_start(
                                hA[ch][:, :, c * 32:(c + 1) * 32],
                                ag_outs[ch][c * 512:(c + 1) * 512].rearrange(
                                    "(a p) n -> p a n", p=128))

            # ---------------- fc tail ----------------
            while fc_row[0] < 2 * (NCH - 1):
                fc_unit()
            ch = NCH - 1
            for c in range(NCORES):
                engs[c % 3].dma_start(
                    hA[ch][:, :, c * 32:(c + 1) * 32],
                    ag_outs[ch][c * 512:(c + 1) * 512].rearrange(
                        "(a p) n -> p a n", p=128))
            while fc_row[0] < NROW:
                fc_unit()

    nc.compile()
    return nc


def _prep(inputs, n_pad):
    import ml_dtypes
    bf = ml_dtypes.bfloat16
    NB = n_pad // 4
    gf = np.ascontiguousarray(np.asarray(inputs["graph_features"], np.float32))
    nf = np.ascontiguousarray(np.asarray(inputs["node_features"], np.float32))
    emb = np.asarray(inputs["embedding"], np.float32)
    W_a = np.asarray(inputs["W_a"], np.float32)
    b_a = np.asarray(inputs["b_a"], np.float32)
    W_c = np.asarray(inputs["W_c"], np.float32)
    b_c = np.asarray(inputs["b_c"], np.float32)
    W_ih0 = np.asarray(inputs["W_ih0"], np.float32)
    W_hh0 = np.asarray(inputs["W_hh0"], np.float32)
    b0 = np.asarray(inputs["b_ih0"], np.float32) + np.asarray(inputs["b_hh0"], np.float32)
    W_ih1 = np.asarray(inputs["W_ih1"], np.float32)
    W_hh1 = np.asarray(inputs["W_hh1"], np.float32)
    b1 = np.asarray(inputs["b_ih1"], np.float32) + np.asarray(inputs["b_hh1"], np.float32)
    W_fc = np.asarray(inputs["W_fc"], np.float32)
    bidx = np.asarray(inputs["batch_idx"]).astype(np.int64)
    caps = np.asarray(inputs["captions"]).astype(np.int64)

    # stationary-slot permutation: slot (partition p, k-chunk c) <-> h index
    # PERM = 128*(p//32) + 32*c + (p%32); xperm[c*128+p] = PERM(p, c)
    cc, pp = np.meshgrid(np.arange(4), np.arange(128), indexing="ij")
    xperm = (128 * (pp // 32) + 32 * cc + (pp % 32)).reshape(-1)  # [512]
    # banded gate-column order: stored col j*512+g*128+q <-> g*512+128j+q
    ss = np.arange(G4)
    jj, rr = ss // 512, ss % 512
    colmap = (rr // 128) * 512 + 128 * jj + (rr % 128)

    # gate scale: i,f,o gates halved (sigmoid-via-tanh); g full.
    gsc = np.ones((G4,), np.float32) * 0.5
    gsc[2 * H:3 * H] = 1.0        # g gate (order i,f,g,o)
    # h-doubling: consumers of h scale by 0.5
    w0 = np.concatenate([W_ih0[:, 512:].T * gsc[None, :],
                         W_hh0.T * (0.5 * gsc)[None, :]], 0)
    w1 = np.concatenate([W_ih1.T * (0.5 * gsc)[None, :],
                         W_hh1.T * (0.5 * gsc)[None, :]], 0)
    wemb = W_ih0[:, :512].T * gsc[None, :]
    b0s = b0 * gsc
    b1s = b1 * gsc
    # apply gate-col banding; x-row permutation per 512-row group
    w0 = w0[:, colmap]
    w0 = np.concatenate([w0[xperm], w0[512 + xperm]], 0)
    w1 = w1[:, colmap]
    w1 = np.concatenate([w1[xperm], w1[512 + xperm]], 0)
    wemb = wemb[:, colmap]
    b0s = b0s[colmap]
    b1s = b1s[colmap]

    def blocks(a):
        K, N = a.shape
        return np.ascontiguousarray(a.reshape(K // 128, 128, N).transpose(1, 0, 2))

    wcT_full = np.zeros((640, 512), np.float32)
    wcT_full[:512] = W_c.T
    wcT_full[512] = b_c
    wca_full = np.zeros((640, 512), np.float32)
    wca_full[:512, :] = 0.5 * (W_c.T @ W_a)[:, xperm]
    wca_full[512, :] = 0.5 * (b_c @ W_a)[xperm]
    i8p = np.zeros((128, 8), np.float32)
    i8p[:8, :8] = np.eye(8)
    mi32 = np.zeros((128, 32), np.float32)
    mi32[:8, :8] = np.eye(8)
    mi32[8, :] = 1.0
    bsum = np.zeros((128, 128), np.float32)
    kk, mm = np.meshgrid(np.arange(128), np.arange(128), indexing="ij")
    bsum[(kk % 32 == mm % 32) & (kk % 32 < 8)] = 1.0
    b0c = np.tile(b0s[None, :], (128, 1)).astype(np.float32)
    b1r = np.tile(b1s[None, :], (8, 1))
    use_b0 = bool(np.any(b0 != 0))
    use_b1 = bool(np.any(b1 != 0))
    sb_ba = (nf @ W_c.T + b_c) @ b_a      # per-node b_a fold for scores

    maps = []
    for k in range(NCORES):
        sel = (bidx >= k * BL) & (bidx < (k + 1) * BL)
        nodes = np.nonzero(sel)[0]
        cnt = len(nodes)
        # band-major node layout: band j holds nodes[off_j:off_j+s_j]
        s_j = [cnt // 4 + (1 if j < cnt % 4 else 0) for j in range(4)]
        off = np.cumsum([0] + s_j)
        stored = np.full((n_pad,), -1, np.int64)
        for j in range(4):
            stored[j * NB:j * NB + s_j[j]] = nodes[off[j]:off[j + 1]]
        valid = stored >= 0
        # host phase A: NP (band-major stored rows), NPa = 0.5*NP@W_a
        NP = np.zeros((n_pad, H), np.float32)
        NP[valid] = nf[stored[valid]] @ W_c.T + b_c
        npTa = 0.5 * (NP @ W_a)                        # [n_pad, H]
        msk_full = np.zeros((128, n_pad), np.float32)
        msk_full[8, :] = NEG
        msk_full[8, valid] += NP[valid] @ b_a
        lb = bidx[stored[valid]] - k * BL
        msk_full[lb, np.nonzero(valid)[0]] = -NEG
        npTi = np.zeros((128, 5, n_pad), np.float32)
        for mt in range(4):
            npTi[:, mt, :] = npTa.T[xperm[mt * 128:(mt + 1) * 128], :]
        npTi[:, 4, :] = msk_full
        s2 = np.arange(n_pad)
        rr2 = s2 % 128
        bsrc = (rr2 // 32) * NB + (s2 // 128) * 32 + (rr2 % 32)
        nkc = NB // 32
        npBi = NP[bsrc].reshape(nkc, 128, H).transpose(1, 0, 2)
        # host EG0: emb lookups @ W_ih0[:, :512] (+ b0), banded gate cols
        e = emb[caps[k * BL:(k + 1) * BL]]             # [8, T, E]
        eg = np.ascontiguousarray(
            e.transpose(1, 0, 2).reshape(POS, E)) @ wemb
        if use_b0:
            eg = eg + b0s[None, :]
        wfc = np.zeros((VPAD, H), np.float32)
        wfc[:VSH] = 0.5 * W_fc[k * VSH:(k + 1) * VSH]
        wfcb = blocks(np.ascontiguousarray(wfc.T[xperm]))  # [128, 4, VPAD]
        m = {
            "npTi": npTi.astype(bf),
            "npBi": np.ascontiguousarray(npBi).astype(bf),
            "egd": eg.astype(bf),
            "mi32": mi32.astype(bf),
            "i8p": i8p.astype(bf), "bsum": bsum,
            "w0T": blocks(w0).astype(bf), "w1T": blocks(w1).astype(bf),
            "b1r": b1r.astype(bf),
            "gfT": blocks(np.ascontiguousarray(
                2.0 * gf[k * BL:(k + 1) * BL].T[xperm])).astype(bf),
            "wfcb": wfcb.astype(bf),
        }
        maps.append(m)
    return maps, use_b0, use_b1


def kernel(**inputs) -> np.ndarray:
    from concourse.bass_utils import run_bass_kernel_spmd

    bidx = np.asarray(inputs["batch_idx"]).astype(np.int64)
    counts = np.bincount(bidx // BL, minlength=NCORES)
    n_pad = max(256, int(math.ceil(counts.max() / 128.0)) * 128)
    assert n_pad // 4 <= 480, f"band width {n_pad // 4} exceeds PSUM bank"
    maps, use_b0, use_b1 = _prep(inputs, n_pad)
    key = (n_pad, use_b0, use_b1)
    if key not in _COMPILED:
        _COMPILED[key] = _build(n_pad, use_b0, use_b1)
    res = run_bass_kernel_spmd(_COMPILED[key], maps,
                               core_ids=list(range(NCORES)))
    b_fc = np.asarray(inputs["b_fc"], np.float32)
    out = np.empty((B, T, V), np.float32)
    for k in range(NCORES):
        o = np.asarray(res.results[k]["out"]).astype(np.float32)
        # [ch, pc, c4, dt, b, v] -> [pc, c4, b, ch, dt, v]
        o = o.reshape(NCH, 2, 4, 4, 8, VPAD).transpose(1, 2, 4, 0, 3, 5)
        out[:, :, k * VSH:(k + 1) * VSH] = o.reshape(B, T, VPAD)[:, :, :VSH]
    if np.any(b_fc != 0):
        out += b_fc[None, None, :]
    return out


# revision 27
# speedup vs baseline: 2.0614x; 1.2538x over previous
"""AttentionLSTMDecoder Trainium2 kernel (8-core SPMD), v3.

Sharding: data-parallel over batch B=64 -> 8 graphs/core for the
recurrent part (attention over that core's node segment + 2-layer LSTM),
AllGather of h1 trajectories in 4-step chunks (bf16), vocab-sharded fc
(each core computes a 4096-wide padded vocab slice for all positions).

v3 changes vs v2:
- ALL recurrence matmuls use 4x PE column-tiling: the four quarters of
  every M=8 matmul run concurrently in independent 128x32 PE tiles
  (outputs at PSUM partitions 32j), ~4x faster streaming.
- "banded" data layout: band j (partitions 32j..32j+8) holds batch rows
  for the j-th 128-wide h-slice. All ACT/DVE elementwise work becomes
  [128, 128..512]-shaped (one instruction instead of four, 4x less
  free-dim per lane).
- DVE 32x32 block transposes (vector.transpose) replace PE transposes;
  a fixed h-permutation PERM(p,c) = 128*(p//32)+32*c+(p%32), applied
  host-side to all weight matrices, makes the block-transposed banded
  tensors directly usable as matmul stationaries.
- segment-softmax denominator via tiny per-band column-sum matmuls.
- fc also issued as 4 concurrent 32-row band matmuls (same tile mode,
  no PE mode switches inside the loop).
"""

import math

import numpy as np

B, T, H, E, V, NTOT = 64, 20, 512, 512, 32000, 8192
NCORES = 8
BL = B // NCORES          # 8 graphs per core
POS = T * BL              # 160 positions per core
VSH = V // NCORES         # 4000 vocab rows per core
VPAD = 4096               # padded vocab shard
G4 = 4 * H                # 2048 gate width
NEG = -40.0               # mask bias for off-segment scores
NCH = (T + 3) // 4        # AllGather chunks (4 steps each)
NROW = NCH * 2            # fc output rows of 128 positions

_COMPILED = {}


def _build(n_pad, use_b0, use_b1):
    import concourse.bacc as bacc
    import concourse.mybir as mybir
    import concourse.tile as tile
    from concourse.alu_op_type import AluOpType
    from contextlib import ExitStack

    f32 = mybir.dt.float32
    bf16 = mybir.dt.bfloat16
    AF = mybir.ActivationFunctionType
    ADD, MULT = AluOpType.add, AluOpType.mult

    NB = n_pad // 4           # nodes per band
    nkc = NB // 32            # 32-node blocks per band = ctx K-chunks
    nck = (n_pad + 511) // 512
    nc = bacc.Bacc("TRN2", target_bir_lowering=False, debug=False,
                   num_devices=NCORES)

    D = {}
    def din(name, shape, dt=bf16):
        D[name] = nc.dram_tensor(name, shape, dt, kind="ExternalInput").ap()
        return D[name]

    npTi = din("npTi", [128, 5, n_pad])  # NPa.T perm chunks + mask block
    npBi = din("npBi", [128, nkc, 512])  # NP rows, chunk-node-major
    egd = din("egd", [POS, G4])          # emb@W_ih0[:, :512] (+b0), banded
    mi32 = din("mi32", [128, 32])        # mask lhsT: I8 + ones row, 32 cols
    i8p = din("i8p", [128, 8])           # identity rows 0-7, zero below
    w0T = din("w0T", [128, 8, G4])             # [W_ctx.T; W_hh0.T] perm/banded
    w1T = din("w1T", [128, 8, G4])             # [W_ih1.T; W_hh1.T] perm/banded
    b1r = din("b1r", [8, G4])            # b1 rows banded cols (only if used)
    gfT = din("gfT", [128, 4, 8])              # 2*graph_features.T perm blocks
    wfcb = din("wfcb", [128, 4, VPAD])         # W_fc.T/2 shard, perm rows
    bsum = din("bsum", [128, 128], f32)  # band-sum matrix (k%32==m%32<8)
    out_d = nc.dram_tensor("out", [NROW, 128, VPAD], bf16,
                           kind="ExternalOutput").ap()

    with tile.TileContext(nc) as tc, ExitStack() as ctx:
        res = ctx.enter_context(tc.tile_pool(name="res", bufs=1))
        dram = ctx.enter_context(tc.tile_pool(name="dram", bufs=1, space="DRAM"))
        drsh = ctx.enter_context(tc.tile_pool(name="drsh", bufs=1, space="DRAM"))

        npT = res.tile([128, 5, n_pad], bf16, tag="npT")   # [NPa.T blocks; mask]
        npB = res.tile([128, nkc, 512], bf16, tag="npB")   # NP chunk-node-major
        i8s = res.tile([128, 8], bf16, tag="i8s")
        msT = res.tile([128, 32], bf16, tag="msT")
        bss = res.tile([128, 128], f32, tag="bss")
        hall = res.tile([128, 4, POS], bf16, tag="hall")
        hT0 = [res.tile([128, 128], bf16, tag=f"hT0{i}", name=f"hT0{i}")
               for i in range(2)]
        hT1 = [res.tile([128, 128], bf16, tag=f"hT1{i}", name=f"hT1{i}")
               for i in range(2)]
        c0s = res.tile([128, 128], f32, tag="c0s")
        c1s = res.tile([128, 128], f32, tag="c1s")
        w0s = res.tile([128, 8, G4], bf16, tag="w0s")
        w1s = res.tile([128, 8, G4], bf16, tag="w1s")
        wfcs = res.tile([128, 4, VPAD], bf16, tag="wfcs")
        egs = [res.tile([128, G4], bf16, tag=f"egs{i}", name=f"egs{i}")
               for i in range(2)]
        hA = [res.tile([128, 4, 256], bf16, tag=f"hA{ch}", name=f"hA{ch}")
              for ch in range(NCH)]
        b1s = res.tile([128, G4], bf16, tag="b1s") if use_b1 else None

        ag_ins = [dram.tile([512, 32], bf16, tag=f"agi{i}", name=f"agi{i}")
                  for i in range(NCH)]
        ag_outs = [drsh.tile([NCORES * 512, 32], bf16,
                             addr_space="Shared", tag=f"ago{i}",
                             name=f"ago{i}")
                   for i in range(NCH)]

        # ---------------- input DMAs (phase A precomputed on host) -----
        engs = [nc.sync, nc.scalar, nc.gpsimd]
        nc.gpsimd.memset(c0s[:], 0.0)
        nc.gpsimd.memset(c1s[:], 0.0)
        nc.gpsimd.memset(egs[0][:], 0.0)
        nc.gpsimd.memset(egs[1][:], 0.0)
        if use_b1:
            nc.gpsimd.memset(b1s[:], 0.0)
            nc.sync.dma_start(b1s[0:8, :], b1r[:])
        nc.gpsimd.dma_start(
            hT0[0][:].rearrange("p (c n) -> p c n", c=4)[:, :, 0:8], gfT[:])
        nc.gpsimd.dma_start(
            hT1[0][:].rearrange("p (c n) -> p c n", c=4)[:, :, 0:8], gfT[:])
        nc.sync.dma_start(i8s[:], i8p[:])
        nc.scalar.dma_start(msT[:], mi32[:])
        nc.sync.dma_start(bss[:], bsum[:])
        n = 0
        for i in range(5):
            engs[n % 3].dma_start(npT[:, i, :], npTi[:, i, :]); n += 1
        for i in (4, 5, 6, 7):
            engs[n % 3].dma_start(w0s[:, i, :], w0T[:, i, :]); n += 1
        for i in range(nkc):
            engs[n % 3].dma_start(npB[:, i, :], npBi[:, i, :]); n += 1
        for i in (4, 5, 6, 7):
            engs[n % 3].dma_start(w1s[:, i, :], w1T[:, i, :]); n += 1
        for i in (0, 1, 2, 3):
            engs[n % 3].dma_start(w0s[:, i, :], w0T[:, i, :]); n += 1
        for i in (0, 1, 2, 3):
            engs[n % 3].dma_start(w1s[:, i, :], w1T[:, i, :]); n += 1
        for i in range(4):
            engs[n % 3].dma_start(wfcs[:, i, :], wfcb[:, i, :]); n += 1

        # ---------------- recurrence + interleaved fc ----------------
        fc_row = [0]      # next output row (ch*2+pc), 0..NROW-1
        fc_vc = [0]       # next vocab chunk within row, 0..7
        fc_cur = [None]   # current fco tile

        with tc.tile_pool(name="stepp", bufs=1) as stepp, \
             tc.tile_pool(name="fco", bufs=2) as fco, \
             tc.tile_pool(name="gp0", bufs=2, space="PSUM") as gp0, \
             tc.tile_pool(name="gp1", bufs=2, space="PSUM") as gp1, \
             tc.tile_pool(name="scp", bufs=1, space="PSUM") as scp, \
             tc.tile_pool(name="ctp", bufs=1, space="PSUM") as ctp, \
             tc.tile_pool(name="fcp", bufs=2, space="PSUM") as fcp:

            def fc_unit():
                """One (row, vc) fc unit: 16 band matmuls + copy; DMA on
                row end."""
                row, vc = fc_row[0], fc_vc[0]
                ch, pc = divmod(row, 2)
                if vc == 0:
                    fc_cur[0] = fco.tile([128, VPAD], bf16, tag="fcr",
                                         name=f"fcr{row}")
                p = fcp.tile([128, 512], f32, tag="fc")
                for kt in range(4):
                    for j in range(4):
                        nc.tensor.matmul(
                            p[32 * j:32 * j + 32, :],
                            hA[ch][:, kt, pc * 128 + 32 * j:pc * 128 + 32 * j + 32],
                            wfcs[:, kt, vc * 512:(vc + 1) * 512],
                            start=(kt == 0), stop=(kt == 3),
                            tile_position=(0, 32 * j))
                if vc % 2 == 0:
                    nc.scalar.copy(fc_cur[0][:, vc * 512:(vc + 1) * 512], p[:])
                else:
                    nc.vector.tensor_copy(
                        fc_cur[0][:, vc * 512:(vc + 1) * 512], p[:])
                fc_vc[0] += 1
                if fc_vc[0] == 4:
                    nc.sync.dma_start(out_d[row][:, 0:2048],
                                      fc_cur[0][:, 0:2048])
                if fc_vc[0] == 8:
                    nc.sync.dma_start(out_d[row][:, 2048:VPAD],
                                      fc_cur[0][:, 2048:VPAD])
                    fc_row[0] += 1
                    fc_vc[0] = 0

            def fc_budget(t, budget):
                if t < 10:
                    avail_rows = 0
                else:
                    avail_rows = min(NROW, 2 * ((t - 8) // 4 + 1))
                while budget > 0 and fc_row[0] < avail_rows:
                    fc_unit()
                    budget -= 1

            def cell(Tg, cS, hTn):
                """Banded LSTM cell from tanh'd gates Tg [128,512]
                (t_i|t_f|t_g|t_o per 128-col block, i,f,o pre-halved);
                updates cS (=2c) in place, writes block-transposed doubled
                hidden state into hTn [128,128]."""
                v = stepp.tile([128, 128], f32, tag="v")
                nc.vector.scalar_tensor_tensor(
                    v[:], Tg[:, 0:128], 1.0, Tg[:, 256:384], ADD, MULT)
                u = stepp.tile([128, 128], f32, tag="u")
                nc.vector.scalar_tensor_tensor(
                    u[:], Tg[:, 128:256], 1.0, cS[:], ADD, MULT)
                nc.vector.scalar_tensor_tensor(
                    cS[:], u[:], 0.5, v[:], MULT, ADD)
                tch = stepp.tile([128, 128], f32, tag="tch")
                nc.scalar.activation(tch[:], cS[:], AF.Tanh, scale=0.5)
                hn = stepp.tile([128, 128], bf16, tag="hn")
                nc.vector.scalar_tensor_tensor(
                    hn[:, 0:64], Tg[:, 384:448], 1.0, tch[:, 0:64], ADD, MULT)
                nc.vector.transpose(hTn[:, 0:64], hn[:, 0:64])
                nc.vector.scalar_tensor_tensor(
                    hn[:, 64:128], Tg[:, 448:512], 1.0, tch[:, 64:128],
                    ADD, MULT)
                nc.vector.transpose(hTn[:, 64:128], hn[:, 64:128])

            nc.sync.dma_start(egs[0][0:8, :], egd[0:8, :])
            for t in range(T):
                h0c = hT0[t % 2][:].rearrange("p (c n) -> p c n", c=4)
                h1c = hT1[t % 2][:].rearrange("p (c n) -> p c n", c=4)
                h0n = hT0[(t + 1) % 2]
                h1n = hT1[(t + 1) % 2]
                eg = egs[t % 2]
                if t + 1 < T:
                    nc.sync.dma_start(egs[(t + 1) % 2][0:8, :],
                                      egd[(t + 1) * 8:(t + 2) * 8, :])

                # g0 pre-run: eg bias + h0-part (no dep on this step's ctx)
                g0P = gp0.tile([128, 512], f32, tag="g0")
                for j in range(4):
                    nc.tensor.matmul(g0P[32 * j:32 * j + 8, :], i8s[:],
                                     eg[:, j * 512:(j + 1) * 512],
                                     start=True, stop=False,
                                     tile_position=(0, 32 * j))
                for c in range(4):
                    for j in range(4):
                        nc.tensor.matmul(g0P[32 * j:32 * j + 8, :],
                                         h0c[:, c, 0:8],
                                         w0s[:, 4 + c, j * 512:(j + 1) * 512],
                                         start=False, stop=False,
                                         tile_position=(0, 32 * j))

                # scores S banded [32j+b, NB] = (mask + H1/2 @ NPa.T)
                scP = scp.tile([128, NB + 32], f32, tag="sc")
                for j in range(4):
                    nc.tensor.matmul(scP[32 * j:32 * j + 32, 0:NB], msT[:],
                                     npT[:, 4, j * NB:(j + 1) * NB],
                                     start=True, stop=False,
                                     tile_position=(0, 32 * j))
                for c in range(4):
                    for j in range(4):
                        nc.tensor.matmul(scP[32 * j:32 * j + 8, 0:NB],
                                         h1c[:, c, 0:8],
                                         npT[:, c, j * NB:(j + 1) * NB],
                                         start=False, stop=(c == 3),
                                         tile_position=(0, 32 * j))
                fc_budget(t, 3)

                NBa = (nkc // 2) * 32
                Eta = stepp.tile([128, NBa], bf16, tag="Eta", bufs=2)
                Etb = stepp.tile([128, NB - NBa], bf16, tag="Etb", bufs=2)
                dp = stepp.tile([128, 2], f32, tag="dp")
                nc.scalar.activation(Eta[:], scP[:, 0:NBa], AF.Exp,
                                     accum_out=dp[:, 0:1])
                nc.scalar.activation(Etb[:], scP[:, NBa:NB], AF.Exp,
                                     accum_out=dp[:, 1:2])
                # e.T via DVE 32x32 block transposes (pipelined halves)
                eTa = stepp.tile([128, NBa], bf16, tag="eTa", bufs=2)
                eTb = stepp.tile([128, NB - NBa], bf16, tag="eTb", bufs=2)
                nc.vector.transpose(eTa[:], Eta[:])
                nc.vector.transpose(eTb[:], Etb[:])
                # den = band-sum of dp halves, replicated; recip
                for j in range(4):
                    nc.tensor.matmul(scP[32 * j:32 * j + 32, NB:NB + 1],
                                     bss[:, 32 * j:32 * j + 32], dp[:, 0:1],
                                     start=True, stop=False,
                                     tile_position=(0, 32 * j))
                for j in range(4):
                    nc.tensor.matmul(scP[32 * j:32 * j + 32, NB:NB + 1],
                                     bss[:, 32 * j:32 * j + 32], dp[:, 1:2],
                                     start=False, stop=True,
                                     tile_position=(0, 32 * j))
                r = stepp.tile([128, 1], f32, tag="r")
                nc.vector.reciprocal(r[:], scP[:, NB:NB + 1])

                eTav = eTa[:].rearrange("p (k n) -> p k n", k=nkc // 2)
                eTbv = eTb[:].rearrange("p (k n) -> p k n", k=nkc - nkc // 2)
                ctxP = ctp.tile([128, 128], f32, tag="cx")
                for kc in range(nkc):
                    ev = (eTav[:, kc, 0:8] if kc < nkc // 2
                          else eTbv[:, kc - nkc // 2, 0:8])
                    for j in range(4):
                        nc.tensor.matmul(ctxP[32 * j:32 * j + 8, :],
                                         ev,
                                         npB[:, kc, j * 128:(j + 1) * 128],
                                         start=(kc == 0), stop=(kc == nkc - 1),
                                         tile_position=(0, 32 * j))
                ctxS = stepp.tile([128, 128], bf16, tag="ctxS")
                nc.vector.tensor_scalar_mul(ctxS[:], ctxP[:], r[:])
                ctxT = stepp.tile([128, 128], bf16, tag="ctxT", bufs=2)
                nc.vector.transpose(ctxT[:, 0:64], ctxS[:, 0:64])
                nc.vector.transpose(ctxT[:, 64:128], ctxS[:, 64:128])
                ctv = ctxT[:].rearrange("p (c n) -> p c n", c=4)

                # close gates0 with ctx-part; tanh evac; cell0
                for c in range(4):
                    for j in range(4):
                        nc.tensor.matmul(g0P[32 * j:32 * j + 8, :],
                                         ctv[:, c, 0:8],
                                         w0s[:, c, j * 512:(j + 1) * 512],
                                         start=False, stop=(c == 3),
                                         tile_position=(0, 32 * j))
                Tg0 = stepp.tile([128, 512], f32, tag="Tg0")
                nc.scalar.activation(Tg0[:, 0:384], g0P[:, 0:384], AF.Tanh)
                nc.scalar.activation(Tg0[:, 384:512], g0P[:, 384:512], AF.Tanh)

                # g1 pre-run: h1-part (+b1), overlaps cell0
                g1P = gp1.tile([128, 512], f32, tag="g1")
                if use_b1:
                    for j in range(4):
                        nc.tensor.matmul(g1P[32 * j:32 * j + 8, :], i8s[:],
                                         b1s[:, j * 512:(j + 1) * 512],
                                         start=True, stop=False,
                                         tile_position=(0, 32 * j))
                for c in range(4):
                    for j in range(4):
                        nc.tensor.matmul(g1P[32 * j:32 * j + 8, :],
                                         h1c[:, c, 0:8],
                                         w1s[:, 4 + c, j * 512:(j + 1) * 512],
                                         start=(c == 0 and not use_b1),
                                         stop=False,
                                         tile_position=(0, 32 * j))

                cell(Tg0, c0s, h0n)
                h0nv = h0n[:].rearrange("p (c n) -> p c n", c=4)

                fc_budget(t, 1)

                # close gates1 with h0n-part; tanh evac; cell1
                for c in range(4):
                    for j in range(4):
                        nc.tensor.matmul(g1P[32 * j:32 * j + 8, :],
                                         h0nv[:, c, 0:8],
                                         w1s[:, c, j * 512:(j + 1) * 512],
                                         start=False, stop=(c == 3),
                                         tile_position=(0, 32 * j))
                Tg1 = stepp.tile([128, 512], f32, tag="Tg1")
                nc.scalar.activation(Tg1[:, 0:384], g1P[:, 0:384], AF.Tanh)
                nc.scalar.activation(Tg1[:, 384:512], g1P[:, 384:512], AF.Tanh)

                fc_budget(t, 2)

                cell(Tg1, c1s, h1n)
                nc.vector.tensor_copy(
                    hall[:, :, t * 8:(t + 1) * 8],
                    h1n[:].rearrange("p (c n) -> p c n", c=4)[:, :, 0:8])

                if t % 4 == 3:
                    ch = t // 4
                    agi = ag_ins[ch]
                    nc.gpsimd.dma_start(
                        agi[:].rearrange("(a p) n -> p a n", p=128),
                        hall[:, :, ch * 32:(ch + 1) * 32])
                    nc.gpsimd.collective_compute(
                        "AllGather", mybir.AluOpType.bypass,
                        replica_groups=[list(range(NCORES))],
                        ins=[agi.opt()], outs=[ag_outs[ch].opt()])
                    for c in range(NCORES):
                        nc.gpsimd.dma_start(
                            hA[ch][:, :, c * 32:(c + 1) * 32],
                            ag_outs[ch][c * 512:(c + 1) * 512].rearrange(
                                "(a p) n -> p a n", p=128))

            # ---------------- fc tail ----------------
            while fc_row[0] < NROW:
                fc_unit()

    nc.compile()
    return nc


def _prep(inputs, n_pad):
    import ml_dtypes
    bf = ml_dtypes.bfloat16
    NB = n_pad // 4
    gf = np.ascontiguousarray(np.asarray(inputs["graph_features"], np.float32))
    nf = np.ascontiguousarray(np.asarray(inputs["node_features"], np.float32))
    emb = np.asarray(inputs["embedding"], np.float32)
    W_a = np.asarray(inputs["W_a"], np.float32)
    b_a = np.asarray(inputs["b_a"], np.float32)
    W_c = np.asarray(inputs["W_c"], np.float32)
    b_c = np.asarray(inputs["b_c"], np.float32)
    W_ih0 = np.asarray(inputs["W_ih0"], np.float32)
    W_hh0 = np.asarray(inputs["W_hh0"], np.float32)
    b0 = np.asarray(inputs["b_ih0"], np.float32) + np.asarray(inputs["b_hh0"], np.float32)
    W_ih1 = np.asarray(inputs["W_ih1"], np.float32)
    W_hh1 = np.asarray(inputs["W_hh1"], np.float32)
    b1 = np.asarray(inputs["b_ih1"], np.float32) + np.asarray(inputs["b_hh1"], np.float32)
    W_fc = np.asarray(inputs["W_fc"], np.float32)
    bidx = np.asarray(inputs["batch_idx"]).astype(np.int64)
    caps = np.asarray(inputs["captions"]).astype(np.int64)

    # stationary-slot permutation: slot (partition p, k-chunk c) <-> h index
    # PERM = 128*(p//32) + 32*c + (p%32); xperm[c*128+p] = PERM(p, c)
    cc, pp = np.meshgrid(np.arange(4), np.arange(128), indexing="ij")
    xperm = (128 * (pp // 32) + 32 * cc + (pp % 32)).reshape(-1)  # [512]
    # banded gate-column order: stored col j*512+g*128+q <-> g*512+128j+q
    ss = np.arange(G4)
    jj, rr = ss // 512, ss % 512
    colmap = (rr // 128) * 512 + 128 * jj + (rr % 128)

    # gate scale: i,f,o gates halved (sigmoid-via-tanh); g full.
    gsc = np.ones((G4,), np.float32) * 0.5
    gsc[2 * H:3 * H] = 1.0        # g gate (order i,f,g,o)
    # h-doubling: consumers of h scale by 0.5
    w0 = np.concatenate([W_ih0[:, 512:].T * gsc[None, :],
                         W_hh0.T * (0.5 * gsc)[None, :]], 0)
    w1 = np.concatenate([W_ih1.T * (0.5 * gsc)[None, :],
                         W_hh1.T * (0.5 * gsc)[None, :]], 0)
    wemb = W_ih0[:, :512].T * gsc[None, :]
    b0s = b0 * gsc
    b1s = b1 * gsc
    # apply gate-col banding; x-row permutation per 512-row group
    w0 = w0[:, colmap]
    w0 = np.concatenate([w0[xperm], w0[512 + xperm]], 0)
    w1 = w1[:, colmap]
    w1 = np.concatenate([w1[xperm], w1[512 + xperm]], 0)
    wemb = wemb[:, colmap]
    b0s = b0s[colmap]
    b1s = b1s[colmap]

    def blocks(a):
        K, N = a.shape
        return np.ascontiguousarray(a.reshape(K // 128, 128, N).transpose(1, 0, 2))

    wcT_full = np.zeros((640, 512), np.float32)
    wcT_full[:512] = W_c.T
    wcT_full[512] = b_c
    wca_full = np.zeros((640, 512), np.float32)
    wca_full[:512, :] = 0.5 * (W_c.T @ W_a)[:, xperm]
    wca_full[512, :] = 0.5 * (b_c @ W_a)[xperm]
    i8p = np.zeros((128, 8), np.float32)
    i8p[:8, :8] = np.eye(8)
    mi32 = np.zeros((128, 32), np.float32)
    mi32[:8, :8] = np.eye(8)
    mi32[8, :] = 1.0
    bsum = np.zeros((128, 128), np.float32)
    kk, mm = np.meshgrid(np.arange(128), np.arange(128), indexing="ij")
    bsum[(kk % 32 == mm % 32) & (kk % 32 < 8)] = 1.0
    b0c = np.tile(b0s[None, :], (128, 1)).astype(np.float32)
    b1r = np.tile(b1s[None, :], (8, 1))
    use_b0 = bool(np.any(b0 != 0))
    use_b1 = bool(np.any(b1 != 0))
    sb_ba = (nf @ W_c.T + b_c) @ b_a      # per-node b_a fold for scores

    maps = []
    for k in range(NCORES):
        sel = (bidx >= k * BL) & (bidx < (k + 1) * BL)
        nodes = np.nonzero(sel)[0]
        cnt = len(nodes)
        # band-major node layout: band j holds nodes[off_j:off_j+s_j]
        s_j = [cnt // 4 + (1 if j < cnt % 4 else 0) for j in range(4)]
        off = np.cumsum([0] + s_j)
        stored = np.full((n_pad,), -1, np.int64)
        for j in range(4):
            stored[j * NB:j * NB + s_j[j]] = nodes[off[j]:off[j + 1]]
        valid = stored >= 0
        # host phase A: NP (band-major stored rows), NPa = 0.5*NP@W_a
        NP = np.zeros((n_pad, H), np.float32)
        NP[valid] = nf[stored[valid]] @ W_c.T + b_c
        npTa = 0.5 * (NP @ W_a)                        # [n_pad, H]
        msk_full = np.zeros((128, n_pad), np.float32)
        msk_full[8, :] = NEG
        msk_full[8, valid] += NP[valid] @ b_a
        lb = bidx[stored[valid]] - k * BL
        msk_full[lb, np.nonzero(valid)[0]] = -NEG
        npTi = np.zeros((128, 5, n_pad), np.float32)
        for mt in range(4):
            npTi[:, mt, :] = npTa.T[xperm[mt * 128:(mt + 1) * 128], :]
        npTi[:, 4, :] = msk_full
        s2 = np.arange(n_pad)
        rr2 = s2 % 128
        bsrc = (rr2 // 32) * NB + (s2 // 128) * 32 + (rr2 % 32)
        nkc = NB // 32
        npBi = NP[bsrc].reshape(nkc, 128, H).transpose(1, 0, 2)
        # host EG0: emb lookups @ W_ih0[:, :512] (+ b0), banded gate cols
        e = emb[caps[k * BL:(k + 1) * BL]]             # [8, T, E]
        eg = np.ascontiguousarray(
            e.transpose(1, 0, 2).reshape(POS, E)) @ wemb
        if use_b0:
            eg = eg + b0s[None, :]
        wfc = np.zeros((VPAD, H), np.float32)
        wfc[:VSH] = 0.5 * W_fc[k * VSH:(k + 1) * VSH]
        wfcb = blocks(np.ascontiguousarray(wfc.T[xperm]))  # [128, 4, VPAD]
        m = {
            "npTi": npTi.astype(bf),
            "npBi": np.ascontiguousarray(npBi).astype(bf),
            "egd": eg.astype(bf),
            "mi32": mi32.astype(bf),
            "i8p": i8p.astype(bf), "bsum": bsum,
            "w0T": blocks(w0).astype(bf), "w1T": blocks(w1).astype(bf),
            "b1r": b1r.astype(bf),
            "gfT": blocks(np.ascontiguousarray(
                2.0 * gf[k * BL:(k + 1) * BL].T[xperm])).astype(bf),
            "wfcb": wfcb.astype(bf),
        }
        maps.append(m)
    return maps, use_b0, use_b1


def kernel(**inputs) -> np.ndarray:
    from concourse.bass_utils import run_bass_kernel_spmd

    bidx = np.asarray(inputs["batch_idx"]).astype(np.int64)
    counts = np.bincount(bidx // BL, minlength=NCORES)
    n_pad = max(256, int(math.ceil(counts.max() / 128.0)) * 128)
    assert n_pad // 4 <= 480, f"band width {n_pad // 4} exceeds PSUM bank"
    maps, use_b0, use_b1 = _prep(inputs, n_pad)
    key = (n_pad, use_b0, use_b1)
    if key not in _COMPILED:
        _COMPILED[key] = _build(n_pad, use_b0, use_b1)
    res = run_bass_kernel_spmd(_COMPILED[key], maps,
                               core_ids=list(range(NCORES)))
    b_fc = np.asarray(inputs["b_fc"], np.float32)
    out = np.empty((B, T, V), np.float32)
    for k in range(NCORES):
        o = np.asarray(res.results[k]["out"]).astype(np.float32)
        # [ch, pc, c4, dt, b, v] -> [pc, c4, b, ch, dt, v]
        o = o.reshape(NCH, 2, 4, 4, 8, VPAD).transpose(1, 2, 4, 0, 3, 5)
        out[:, :, k * VSH:(k + 1) * VSH] = o.reshape(B, T, VPAD)[:, :, :VSH]
    if np.any(b_fc != 0):
        out += b_fc[None, None, :]
    return out
